# revision 45
# baseline (speedup 1.0000x reference)
"""Sparse expert-parallel MoE kernel for Trainium2 (8 NeuronCores).

Strategy (hardcoded for the nn_MoE problem: H=1024, E=8, top-k=2, I=1408,
shared-I=2816, T=2*2048=4096 tokens, f32 inputs):

- The gate (softmax top-2) is tiny (0.03% of FLOPs) and is evaluated on the
  host in float64; routing decisions match the f32 reference (min rank-2/3
  score gap for this problem's data is ~4e-5, far above f32 noise).
- Routed experts are EXPERT-PARALLEL with true top-2 sparsity: core r owns
  expert r and computes it only over the tokens routed to it (host-side
  gather, padded to capacity C = max expert load, split into near-equal
  token chunks <= 512).  This is ~4x fewer FLOPs than dense all-expert
  compute.
- The shared expert is sharded 4x2: cores are split into 4 token-groups of
  2; within a group each core owns a 1408-wide half of the 2816 shared
  intermediate dim (11 full 128-tiles -> no partial-tile waste).  Partials
  are summed on the host.
- Combine: host scatter-adds  w_e(t) * expert_e(x_t)  (f32) plus the shared
  partial sums.  No on-device collectives.
- All matmuls run in bf16 with f32 PSUM accumulation (host pre-casts);
  outputs are stored bf16 and accumulated f32 on the host.
- DMA cost is ~(27ns + 29ns/KB) per line (= per partition per transfer),
  descriptors sprayed round-robin over 16 rings, so everything is packed
  into few transfers of long contiguous per-partition lines, SBUF-resident,
  issued in consumption order (the first matmul group only waits for
  ~1.5 MB).  The shared phase runs first: its startup inputs are smallest.

Layouts put features on the partition axis and tokens on the free axis:
    up:   hg[i, t] = sum_h wg[h, i] * xT[h, t]   (lhsT=wg nat., rhs=xT)
    down: eo[h, t] = sum_i wd[i, h] * act[i, t]  (lhsT=wd nat., rhs=act)
"""

import os
import sys

for _p in ("/opt/trn_rl_repo", "/root/.axon_site/_ro/trn_rl_repo"):
    if os.path.isdir(_p) and _p not in sys.path:
        sys.path.insert(0, _p)

import numpy as np

import concourse.bass as bass
import concourse.mybir as mybir
import concourse.tile as tile
from concourse import bacc
from concourse.bass_utils import run_bass_kernel_spmd

F32 = mybir.dt.float32
BF16 = mybir.dt.bfloat16
BF16_NP = mybir.dt.np(mybir.dt.bfloat16)
AX = mybir.AxisListType
ALU = mybir.AluOpType
ACTF = mybir.ActivationFunctionType

H = 1024           # hidden
E = 8              # experts = cores
TOP_K = 2
I_R = 1408         # routed intermediate = shared intermediate half (2816/2)
TS = 1024          # shared-expert tokens per core (4096 / 4 groups)
N_CORES = 8
KC = H // 128      # 8 contraction chunks over hidden
IT_R = I_R // 128  # 11 intermediate tiles (routed and shared-half alike)
TC = 512           # token tile (PSUM bank = 512 f32)
SH_CHUNKS = (384, 384, 256)  # shared-phase token chunking (see build_nc)

LAST_RESULT = None  # BassKernelResults of the most recent run (for profiling)


def _chunks_of(n):
    """Split n into near-equal chunks <= TC (all big enough to keep the
    PE stream-bound rather than weight-load-bound)."""
    nch = max(1, -(-n // TC))
    base, rem = divmod(n, nch)
    return [base + 1] * rem + [base] * (nch - rem)


def build_nc(C, trace_sim=False, silu_via_sigmoid=False):
    """Build the SPMD Bass program (identical on all 8 cores).

    C: routed-token capacity per core (= max expert load for this input).
    silu_via_sigmoid: CoreSim has no Silu LUT; emulate as x*sigmoid(x).
    """
    nc = bacc.Bacc("TRN2", target_bir_lowering=False, debug=False,
                   num_devices=N_CORES)

    # DMA cost is ~27ns + 29ns/KB PER LINE (= per partition per dma_start),
    # descriptors sprayed round-robin over 16 rings -> pack everything into
    # as few, as-long-as-possible contiguous per-partition lines as we can.
    # x inputs packed [128, KC, ntok], loaded with ONE full-tensor DMA.
    xr = nc.dram_tensor("xr", [128, KC, C], BF16, kind="ExternalInput")
    # shared x in per-chunk pieces (contiguous long lines); the first chunk
    # is 384 tokens so the PE starts after only ~1.25 MB of critical DMA,
    # small enough to start early but big enough that sgu tile arrival
    # stays ahead of consumption
    SH_CH = SH_CHUNKS
    xs_d = [nc.dram_tensor(f"xs{i}", [128, KC, n], BF16,
                           kind="ExternalInput")
            for i, n in enumerate(SH_CH)]
    # routed gate+up combined: rows h, cols [gate I_R | up I_R] per k-slice
    wgu = nc.dram_tensor("wgu", [H, 2 * I_R], BF16, kind="ExternalInput")
    # down weights packed [128, IT, H] (partition = i % 128)
    wd = nc.dram_tensor("wd", [128, IT_R, H], BF16, kind="ExternalInput")
    # shared gate+up packed [128, IT, 2*KC*128]: per-it DMA, 4KB lines
    sgu = nc.dram_tensor("sgu", [128, IT_R, 2 * KC * 128], BF16,
                         kind="ExternalInput")
    sd = nc.dram_tensor("sd", [128, IT_R, H], BF16, kind="ExternalInput")
    # bf16 stores (host accumulates in f32; |y| ~ O(1), rel-err budget 2e-2)
    yr = nc.dram_tensor("yr", [H, C], BF16, kind="ExternalOutput")
    ys = nc.dram_tensor("ys", [H, TS], BF16, kind="ExternalOutput")

    with tile.TileContext(nc, trace_sim=trace_sim) as tc:
        with (
            tc.tile_pool(name="const", bufs=1) as cpool,
            tc.tile_pool(name="act", bufs=2) as actpool,
            tc.tile_pool(name="tmp", bufs=3) as tpool,
            tc.tile_pool(name="eo", bufs=3) as eopool,
            tc.tile_pool(name="ps_up", bufs=4, space="PSUM") as ps_up,
            tc.tile_pool(name="ps_o", bufs=3, space="PSUM") as ps_o,
            tc.tile_pool(name="ps_w", bufs=1, space="PSUM") as ps_w,
        ):
            # ---- HAM pre-warm: the PE sits idle ~7-14us while the first
            # inputs DMA in; a scratch matmul stream un-throttles the PE
            # clock (1.2 -> 2.4 GHz) before real work arrives, and ends
            # close enough to it that the MID window can't re-throttle ----
            scr = cpool.tile([128, 128], BF16, tag="scr")
            nc.vector.memset(scr[:, :], 0.0)
            ps_scr = ps_w.tile([128, 128], F32, tag="warm")
            for _ in range(92):
                nc.tensor.matmul(ps_scr[:64, :64], scr[:, :64], scr[:, :64],
                                 start=True, stop=True)

            # ---- inputs in consumption order; the first psum group needs
            # sgu tile 0 + xs chunk 0, so those go first, then strictly by
            # first-use time (supply rate ~0.45 MB/us must stay ahead of
            # the up-group consumption pace) ----
            sgu_ts = []
            for it in range(IT_R):
                sgut = cpool.tile([128, 2 * KC * 128], BF16, tag=f"sgu{it}")
                sgu_ts.append(sgut)
            xs_ts = []
            for i, n in enumerate(SH_CH):
                xst = cpool.tile([128, KC, n], BF16, tag=f"xs{i}")
                xs_ts.append(xst)
            nc.sync.dma_start(sgu_ts[0][:, :], sgu[:, 0, :])
            nc.sync.dma_start(xs_ts[0][:, :, :], xs_d[0][:, :, :])
            for it in (1, 2):
                nc.sync.dma_start(sgu_ts[it][:, :], sgu[:, it, :])
            nc.sync.dma_start(xs_ts[1][:, :, :], xs_d[1][:, :, :])
            for it in (3, 4):
                nc.sync.dma_start(sgu_ts[it][:, :], sgu[:, it, :])
            nc.sync.dma_start(xs_ts[2][:, :, :], xs_d[2][:, :, :])
            for it in range(5, IT_R):
                nc.sync.dma_start(sgu_ts[it][:, :], sgu[:, it, :])
            sd_t = cpool.tile([128, IT_R, H], BF16, tag="sd")
            nc.sync.dma_start(sd_t[:, :, :], sd[:, :, :])
            # routed inputs (needed ~120us in; stream behind shared ones)
            xr_t = cpool.tile([128, KC, C], BF16, tag="xr")
            nc.sync.dma_start(xr_t[:, :, :], xr[:, :, :])
            wgu_ks = []
            for k in range(KC):
                wguk = cpool.tile([128, 2 * I_R], BF16, tag=f"wgu{k}")
                nc.sync.dma_start(wguk[:, :], wgu[k * 128:(k + 1) * 128, :])
                wgu_ks.append(wguk)
            wd_t = cpool.tile([128, IT_R, H], BF16, tag="wd")
            nc.sync.dma_start(wd_t[:, :, :], wd[:, :, :])

            def swiglu_chunk(xf, n, gate_f, up_f, act_t):
                """act[i, :n] = silu(gate) * up over this token chunk.

                xf: k -> rhs [128, n];  gate_f/up_f: (it, k) -> lhsT block.
                """
                for it in range(IT_R):
                    pg = ps_up.tile([128, TC], F32, tag="up")
                    for k in range(KC):
                        nc.tensor.matmul(pg[:, :n], gate_f(it, k), xf(k),
                                         start=(k == 0), stop=(k == KC - 1))
                    pu = ps_up.tile([128, TC], F32, tag="up")
                    for k in range(KC):
                        nc.tensor.matmul(pu[:, :n], up_f(it, k), xf(k),
                                         start=(k == 0), stop=(k == KC - 1))
                    sa = tpool.tile([128, TC], F32, tag="sa")
                    if silu_via_sigmoid:
                        nc.scalar.activation(sa[:, :n], pg[:, :n],
                                             ACTF.Sigmoid)
                        nc.vector.tensor_mul(sa[:, :n], sa[:, :n], pg[:, :n])
                    else:
                        nc.scalar.activation(sa[:, :n], pg[:, :n], ACTF.Silu)
                    nc.vector.tensor_mul(act_t[:, it, :n], sa[:, :n],
                                         pu[:, :n])

            def down_chunk(act_t, n, dw_t, out_d, t0):
                for hc in range(KC):
                    h0 = hc * 128
                    po = ps_o.tile([128, TC], F32, tag="o")
                    for it in range(IT_R):
                        nc.tensor.matmul(
                            po[:, :n], dw_t[:, it, h0:h0 + 128],
                            act_t[:, it, :n], start=(it == 0),
                            stop=(it == IT_R - 1))
                    eo = eopool.tile([128, TC], BF16)
                    nc.vector.tensor_copy(eo[:, :n], po[:, :n])
                    nc.sync.dma_start(out_d[h0:h0 + 128, t0:t0 + n], eo[:, :n])

            # ---- shared expert half over this core's token group ----
            t0 = 0
            for ci, n in enumerate(SH_CH):
                act_t = actpool.tile([128, IT_R, TC], BF16, tag="act")
                xst = xs_ts[ci]
                swiglu_chunk(
                    lambda k: xst[:, k, :],
                    n,
                    lambda it, k: sgu_ts[it][:, k * 128:(k + 1) * 128],
                    lambda it, k: sgu_ts[it][:, (KC + k) * 128:
                                             (KC + k + 1) * 128], act_t)
                down_chunk(act_t, n, sd_t, ys, t0)
                t0 += n

            # ---- routed expert over gathered tokens ----
            t0 = 0
            for n in _chunks_of(C):
                act_t = actpool.tile([128, IT_R, TC], BF16, tag="act")
                c0 = t0
                swiglu_chunk(
                    lambda k: xr_t[:, k, c0:c0 + n],
                    n,
                    lambda it, k: wgu_ks[k][:, it * 128:(it + 1) * 128],
                    lambda it, k: wgu_ks[k][:, I_R + it * 128:
                                            I_R + (it + 1) * 128], act_t)
                down_chunk(act_t, n, wd_t, yr, t0)
                t0 += n

    nc.compile()
    return nc


def _route_host(xf, gate_w):
    """Replicate the reference MoEGate exactly (float64 for determinism)."""
    logits = xf.astype(np.float64) @ gate_w.astype(np.float64).T
    m = logits.max(axis=-1, keepdims=True)
    ex = np.exp(logits - m)
    sc = ex / ex.sum(axis=-1, keepdims=True)
    topi = np.argsort(-sc, axis=-1, kind="stable")[:, :TOP_K]   # ties: low idx
    topw = np.take_along_axis(sc, topi, axis=-1)
    topw = topw / (topw.sum(axis=-1, keepdims=True) + 1e-20)    # SCALE = 1.0
    return topi, topw


def _pack_x(xT_bf):
    """[H, ntok] -> [128, KC, ntok] partition-major pack."""
    n = xT_bf.shape[1]
    return np.ascontiguousarray(
        xT_bf.reshape(KC, 128, n).transpose(1, 0, 2))


def _pack_x_cm(xT_bf):
    """[H, TS] -> [128, TS//TC, KC, TC] chunk-major pack."""
    a = xT_bf.reshape(KC, 128, TS // TC, TC)
    return np.ascontiguousarray(a.transpose(1, 2, 0, 3))


def _pack_up_w(w):
    """[H, I_R] -> [128, IT, KC*128] pack."""
    a = np.ascontiguousarray(w).astype(BF16_NP)
    a = a.reshape(KC, 128, IT_R, 128).transpose(1, 2, 0, 3)
    return np.ascontiguousarray(a.reshape(128, IT_R, KC * 128))


def _pack_gu(g, u):
    """Two [H, I_R] up-weights -> [128, IT, 2*KC*128] interleaved pack."""
    gp = _pack_up_w(g)
    up = _pack_up_w(u)
    return np.ascontiguousarray(
        np.concatenate([gp[:, :, None, :], up[:, :, None, :]],
                       axis=2).reshape(128, IT_R, 2 * KC * 128))


def _pack_down_w(w):
    """[I_R, H] -> [128, IT, H] pack (partition = i % 128)."""
    a = np.ascontiguousarray(w).astype(BF16_NP)
    return np.ascontiguousarray(a.reshape(IT_R, 128, H).transpose(1, 0, 2))


_NC_CACHE = {}


def kernel(x, gate_w, wg, wu, wd, swg, swu, swd):
    global LAST_RESULT
    x = np.asarray(x, np.float32)
    B, S, _ = x.shape
    T = B * S
    xf = x.reshape(T, H)

    # ---- host gate + dispatch ----
    topi, topw = _route_host(xf, np.asarray(gate_w, np.float32))
    e_ids = topi.ravel()
    t_ids = np.repeat(np.arange(T), TOP_K)
    w_all = topw.ravel()
    order = np.argsort(e_ids, kind="stable")
    e_sorted = e_ids[order]
    t_sorted = t_ids[order]
    w_sorted = w_all[order]
    counts = np.bincount(e_sorted, minlength=E)
    starts = np.concatenate([[0], np.cumsum(counts)])
    C = max(128, int(counts.max()))

    if C not in _NC_CACHE:
        _NC_CACHE[C] = build_nc(C)
    nc = _NC_CACHE[C]

    xfT_bf = np.ascontiguousarray(xf.T).astype(BF16_NP)   # [H, T]
    wg = np.asarray(wg, np.float32)
    wu = np.asarray(wu, np.float32)
    wd = np.asarray(wd, np.float32)
    swg = np.asarray(swg, np.float32)
    swu = np.asarray(swu, np.float32)
    swd = np.asarray(swd, np.float32)

    in_maps = []
    idx_r = []
    w_r = []
    for r in range(N_CORES):
        lo, hi = starts[r], starts[r + 1]
        idx = t_sorted[lo:hi]
        idx_r.append(idx)
        w_r.append(w_sorted[lo:hi])
        xg = np.zeros((H, C), dtype=BF16_NP)
        xg[:, :len(idx)] = xfT_bf[:, idx]
        g, q = divmod(r, 2)
        xs_sl = xfT_bf[:, g * TS:(g + 1) * TS]
        off = np.concatenate([[0], np.cumsum(SH_CHUNKS)])
        in_maps.append({
            "xr": _pack_x(xg),
            **{f"xs{i}": _pack_x(xs_sl[:, off[i]:off[i + 1]])
               for i in range(len(SH_CHUNKS))},
            "wgu": np.ascontiguousarray(
                np.concatenate([wg[r], wu[r]], axis=1)).astype(BF16_NP),
            "wd": _pack_down_w(wd[r]),
            "sgu": _pack_gu(swg[:, q * I_R:(q + 1) * I_R],
                            swu[:, q * I_R:(q + 1) * I_R]),
            "sd": _pack_down_w(swd[q * I_R:(q + 1) * I_R, :]),
        })

    res = run_bass_kernel_spmd(nc, in_maps, core_ids=list(range(N_CORES)))
    LAST_RESULT = res

    # ---- host combine: shared partial sums + weighted routed scatter ----
    yT = np.zeros((H, T), np.float32)
    for g in range(4):
        np.add(res.results[2 * g]["ys"].astype(np.float32),
               res.results[2 * g + 1]["ys"].astype(np.float32),
               out=yT[:, g * TS:(g + 1) * TS])
    for r in range(N_CORES):
        n = len(idx_r[r])
        if n:
            yT[:, idx_r[r]] += (res.results[r]["yr"][:, :n]
                                .astype(np.float32)
                                * w_r[r][None, :].astype(np.float32))
    return np.ascontiguousarray(yT.T).reshape(B, S, H).astype(np.float32)


# revision 49
# speedup vs baseline: 1.0029x; 1.0029x over previous
"""Sparse expert-parallel MoE kernel for Trainium2 (8 NeuronCores).

Strategy (hardcoded for the nn_MoE problem: H=1024, E=8, top-k=2, I=1408,
shared-I=2816, T=2*2048=4096 tokens, f32 inputs):

- The gate (softmax top-2) is tiny (0.03% of FLOPs) and is evaluated on the
  host in float64; routing decisions match the f32 reference (min rank-2/3
  score gap for this problem's data is ~4e-5, far above f32 noise).
- Routed experts are EXPERT-PARALLEL with true top-2 sparsity: core r owns
  expert r and computes it only over the tokens routed to it (host-side
  gather, padded to capacity C = max expert load, split into near-equal
  token chunks <= 512).  This is ~4x fewer FLOPs than dense all-expert
  compute.
- The shared expert is sharded 4x2: cores are split into 4 token-groups of
  2; within a group each core owns a 1408-wide half of the 2816 shared
  intermediate dim (11 full 128-tiles -> no partial-tile waste).  Partials
  are summed on the host.
- Combine: host scatter-adds  w_e(t) * expert_e(x_t)  (f32) plus the shared
  partial sums.  No on-device collectives.
- All matmuls run in bf16 with f32 PSUM accumulation (host pre-casts);
  outputs are stored bf16 and accumulated f32 on the host.
- DMA cost is ~(27ns + 29ns/KB) per line (= per partition per transfer),
  descriptors sprayed round-robin over 16 rings, so everything is packed
  into few transfers of long contiguous per-partition lines, SBUF-resident,
  issued in consumption order (the first matmul group only waits for
  ~1.5 MB).  The shared phase runs first: its startup inputs are smallest.

Layouts put features on the partition axis and tokens on the free axis:
    up:   hg[i, t] = sum_h wg[h, i] * xT[h, t]   (lhsT=wg nat., rhs=xT)
    down: eo[h, t] = sum_i wd[i, h] * act[i, t]  (lhsT=wd nat., rhs=act)
"""

import os
import sys

for _p in ("/opt/trn_rl_repo", "/root/.axon_site/_ro/trn_rl_repo"):
    if os.path.isdir(_p) and _p not in sys.path:
        sys.path.insert(0, _p)

import numpy as np

import concourse.bass as bass
import concourse.mybir as mybir
import concourse.tile as tile
from concourse import bacc
from concourse.bass_utils import run_bass_kernel_spmd

F32 = mybir.dt.float32
BF16 = mybir.dt.bfloat16
BF16_NP = mybir.dt.np(mybir.dt.bfloat16)
AX = mybir.AxisListType
ALU = mybir.AluOpType
ACTF = mybir.ActivationFunctionType

H = 1024           # hidden
E = 8              # experts = cores
TOP_K = 2
I_R = 1408         # routed intermediate = shared intermediate half (2816/2)
TS = 1024          # shared-expert tokens per core (4096 / 4 groups)
N_CORES = 8
KC = H // 128      # 8 contraction chunks over hidden
IT_R = I_R // 128  # 11 intermediate tiles (routed and shared-half alike)
TC = 512           # token tile (PSUM bank = 512 f32)
SH_CHUNKS = (512, 512)  # shared-phase token chunking (see build_nc)

LAST_RESULT = None  # BassKernelResults of the most recent run (for profiling)


def _chunks_of(n):
    """Split n into near-equal chunks <= TC (all big enough to keep the
    PE stream-bound rather than weight-load-bound)."""
    nch = max(1, -(-n // TC))
    base, rem = divmod(n, nch)
    return [base + 1] * rem + [base] * (nch - rem)


def build_nc(C, trace_sim=False, silu_via_sigmoid=False):
    """Build the SPMD Bass program (identical on all 8 cores).

    C: routed-token capacity per core (= max expert load for this input).
    silu_via_sigmoid: CoreSim has no Silu LUT; emulate as x*sigmoid(x).
    """
    nc = bacc.Bacc("TRN2", target_bir_lowering=False, debug=False,
                   num_devices=N_CORES)

    # DMA cost is ~27ns + 29ns/KB PER LINE (= per partition per dma_start),
    # descriptors sprayed round-robin over 16 rings -> pack everything into
    # as few, as-long-as-possible contiguous per-partition lines as we can.
    # x inputs packed [128, KC, ntok], loaded with ONE full-tensor DMA.
    xr = nc.dram_tensor("xr", [128, KC, C], BF16, kind="ExternalInput")
    # shared x in per-chunk pieces (contiguous long lines); the first chunk
    # is 384 tokens so the PE starts after only ~1.25 MB of critical DMA,
    # small enough to start early but big enough that sgu tile arrival
    # stays ahead of consumption
    SH_CH = SH_CHUNKS
    xs_d = [nc.dram_tensor(f"xs{i}", [128, KC, n], BF16,
                           kind="ExternalInput")
            for i, n in enumerate(SH_CH)]  # per-chunk pieces, long lines
    # routed gate+up combined: rows h, cols [gate I_R | up I_R] per k-slice
    wgu = nc.dram_tensor("wgu", [H, 2 * I_R], BF16, kind="ExternalInput")
    # down weights packed [128, IT, H] (partition = i % 128)
    wd = nc.dram_tensor("wd", [128, IT_R, H], BF16, kind="ExternalInput")
    # shared gate+up packed [128, IT, 2*KC*128]: per-it DMA, 4KB lines
    sgu = nc.dram_tensor("sgu", [128, IT_R, 2 * KC * 128], BF16,
                         kind="ExternalInput")
    sd = nc.dram_tensor("sd", [128, IT_R, H], BF16, kind="ExternalInput")
    # bf16 stores (host accumulates in f32; |y| ~ O(1), rel-err budget 2e-2)
    yr = nc.dram_tensor("yr", [H, C], BF16, kind="ExternalOutput")
    ys = nc.dram_tensor("ys", [H, TS], BF16, kind="ExternalOutput")

    with tile.TileContext(nc, trace_sim=trace_sim) as tc:
        with (
            tc.tile_pool(name="const", bufs=1) as cpool,
            tc.tile_pool(name="act", bufs=2) as actpool,
            tc.tile_pool(name="tmp", bufs=3) as tpool,
            tc.tile_pool(name="eo", bufs=3) as eopool,
            tc.tile_pool(name="ps_up", bufs=4, space="PSUM") as ps_up,
            tc.tile_pool(name="ps_o", bufs=3, space="PSUM") as ps_o,
            tc.tile_pool(name="ps_w", bufs=1, space="PSUM") as ps_w,
        ):
            # ---- HAM pre-warm: the PE sits idle ~7-14us while the first
            # inputs DMA in; a scratch matmul stream un-throttles the PE
            # clock (1.2 -> 2.4 GHz) before real work arrives, and ends
            # close enough to it that the MID window can't re-throttle ----
            scr = cpool.tile([128, 128], BF16, tag="scr")
            nc.vector.memset(scr[:, :], 0.0)
            ps_scr = ps_w.tile([128, 128], F32, tag="warm")
            for _ in range(80):
                nc.tensor.matmul(ps_scr[:, :], scr[:, :], scr[:, :],
                                 start=True, stop=True)

            # ---- inputs in consumption order; the first psum group needs
            # sgu tile 0 + xs chunk 0, so those go first, then strictly by
            # first-use time (supply rate ~0.45 MB/us must stay ahead of
            # the up-group consumption pace) ----
            sgu_ts = []
            for it in range(IT_R):
                sgut = cpool.tile([128, 2 * KC * 128], BF16, tag=f"sgu{it}")
                sgu_ts.append(sgut)
            xs_ts = []
            for i, n in enumerate(SH_CH):
                xst = cpool.tile([128, KC, n], BF16, tag=f"xs{i}")
                xs_ts.append(xst)
            nc.sync.dma_start(sgu_ts[0][:, :], sgu[:, 0, :])
            nc.sync.dma_start(xs_ts[0][:, :, :], xs_d[0][:, :, :])
            for it in (1, 2, 3):
                nc.sync.dma_start(sgu_ts[it][:, :], sgu[:, it, :])
            nc.sync.dma_start(xs_ts[1][:, :, :], xs_d[1][:, :, :])
            for it in range(4, IT_R):
                nc.sync.dma_start(sgu_ts[it][:, :], sgu[:, it, :])
            sd_t = cpool.tile([128, IT_R, H], BF16, tag="sd")
            nc.sync.dma_start(sd_t[:, :, :], sd[:, :, :])
            # routed inputs (needed ~120us in; stream behind shared ones)
            xr_t = cpool.tile([128, KC, C], BF16, tag="xr")
            nc.sync.dma_start(xr_t[:, :, :], xr[:, :, :])
            wgu_ks = []
            for k in range(KC):
                wguk = cpool.tile([128, 2 * I_R], BF16, tag=f"wgu{k}")
                nc.sync.dma_start(wguk[:, :], wgu[k * 128:(k + 1) * 128, :])
                wgu_ks.append(wguk)
            wd_t = cpool.tile([128, IT_R, H], BF16, tag="wd")
            nc.sync.dma_start(wd_t[:, :, :], wd[:, :, :])

            def swiglu_chunk(xf, n, gate_f, up_f, act_t):
                """act[i, :n] = silu(gate) * up over this token chunk.

                xf: k -> rhs [128, n];  gate_f/up_f: (it, k) -> lhsT block.
                """
                for it in range(IT_R):
                    pg = ps_up.tile([128, TC], F32, tag="up")
                    for k in range(KC):
                        nc.tensor.matmul(pg[:, :n], gate_f(it, k), xf(k),
                                         start=(k == 0), stop=(k == KC - 1))
                    pu = ps_up.tile([128, TC], F32, tag="up")
                    for k in range(KC):
                        nc.tensor.matmul(pu[:, :n], up_f(it, k), xf(k),
                                         start=(k == 0), stop=(k == KC - 1))
                    sa = tpool.tile([128, TC], F32, tag="sa")
                    if silu_via_sigmoid:
                        nc.scalar.activation(sa[:, :n], pg[:, :n],
                                             ACTF.Sigmoid)
                        nc.vector.tensor_mul(sa[:, :n], sa[:, :n], pg[:, :n])
                    else:
                        nc.scalar.activation(sa[:, :n], pg[:, :n], ACTF.Silu)
                    nc.vector.tensor_mul(act_t[:, it, :n], sa[:, :n],
                                         pu[:, :n])

            def down_chunk(act_t, n, dw_t, out_d, t0):
                for hc in range(KC):
                    h0 = hc * 128
                    po = ps_o.tile([128, TC], F32, tag="o")
                    for it in range(IT_R):
                        nc.tensor.matmul(
                            po[:, :n], dw_t[:, it, h0:h0 + 128],
                            act_t[:, it, :n], start=(it == 0),
                            stop=(it == IT_R - 1))
                    eo = eopool.tile([128, TC], BF16)
                    nc.vector.tensor_copy(eo[:, :n], po[:, :n])
                    nc.sync.dma_start(out_d[h0:h0 + 128, t0:t0 + n], eo[:, :n])

            # ---- shared expert half over this core's token group ----
            t0 = 0
            for ci, n in enumerate(SH_CH):
                act_t = actpool.tile([128, IT_R, TC], BF16, tag="act")
                xst = xs_ts[ci]
                swiglu_chunk(
                    lambda k: xst[:, k, :],
                    n,
                    lambda it, k: sgu_ts[it][:, k * 128:(k + 1) * 128],
                    lambda it, k: sgu_ts[it][:, (KC + k) * 128:
                                             (KC + k + 1) * 128], act_t)
                down_chunk(act_t, n, sd_t, ys, t0)
                t0 += n

            # ---- routed expert over gathered tokens ----
            t0 = 0
            for n in _chunks_of(C):
                act_t = actpool.tile([128, IT_R, TC], BF16, tag="act")
                c0 = t0
                swiglu_chunk(
                    lambda k: xr_t[:, k, c0:c0 + n],
                    n,
                    lambda it, k: wgu_ks[k][:, it * 128:(it + 1) * 128],
                    lambda it, k: wgu_ks[k][:, I_R + it * 128:
                                            I_R + (it + 1) * 128], act_t)
                down_chunk(act_t, n, wd_t, yr, t0)
                t0 += n

    nc.compile()
    return nc


def _route_host(xf, gate_w):
    """Replicate the reference MoEGate exactly (float64 for determinism)."""
    logits = xf.astype(np.float64) @ gate_w.astype(np.float64).T
    m = logits.max(axis=-1, keepdims=True)
    ex = np.exp(logits - m)
    sc = ex / ex.sum(axis=-1, keepdims=True)
    topi = np.argsort(-sc, axis=-1, kind="stable")[:, :TOP_K]   # ties: low idx
    topw = np.take_along_axis(sc, topi, axis=-1)
    topw = topw / (topw.sum(axis=-1, keepdims=True) + 1e-20)    # SCALE = 1.0
    return topi, topw


def _pack_x(xT_bf):
    """[H, ntok] -> [128, KC, ntok] partition-major pack."""
    n = xT_bf.shape[1]
    return np.ascontiguousarray(
        xT_bf.reshape(KC, 128, n).transpose(1, 0, 2))


def _pack_x_cm(xT_bf):
    """[H, TS] -> [128, TS//TC, KC, TC] chunk-major pack."""
    a = xT_bf.reshape(KC, 128, TS // TC, TC)
    return np.ascontiguousarray(a.transpose(1, 2, 0, 3))


def _pack_up_w(w):
    """[H, I_R] -> [128, IT, KC*128] pack."""
    a = np.ascontiguousarray(w).astype(BF16_NP)
    a = a.reshape(KC, 128, IT_R, 128).transpose(1, 2, 0, 3)
    return np.ascontiguousarray(a.reshape(128, IT_R, KC * 128))


def _pack_gu(g, u):
    """Two [H, I_R] up-weights -> [128, IT, 2*KC*128] interleaved pack."""
    gp = _pack_up_w(g)
    up = _pack_up_w(u)
    return np.ascontiguousarray(
        np.concatenate([gp[:, :, None, :], up[:, :, None, :]],
                       axis=2).reshape(128, IT_R, 2 * KC * 128))


def _pack_down_w(w):
    """[I_R, H] -> [128, IT, H] pack (partition = i % 128)."""
    a = np.ascontiguousarray(w).astype(BF16_NP)
    return np.ascontiguousarray(a.reshape(IT_R, 128, H).transpose(1, 0, 2))


_NC_CACHE = {}


def kernel(x, gate_w, wg, wu, wd, swg, swu, swd):
    global LAST_RESULT
    x = np.asarray(x, np.float32)
    B, S, _ = x.shape
    T = B * S
    xf = x.reshape(T, H)

    # ---- host gate + dispatch ----
    topi, topw = _route_host(xf, np.asarray(gate_w, np.float32))
    e_ids = topi.ravel()
    t_ids = np.repeat(np.arange(T), TOP_K)
    w_all = topw.ravel()
    order = np.argsort(e_ids, kind="stable")
    e_sorted = e_ids[order]
    t_sorted = t_ids[order]
    w_sorted = w_all[order]
    counts = np.bincount(e_sorted, minlength=E)
    starts = np.concatenate([[0], np.cumsum(counts)])
    C = max(128, int(counts.max()))

    if C not in _NC_CACHE:
        _NC_CACHE[C] = build_nc(C)
    nc = _NC_CACHE[C]

    xfT_bf = np.ascontiguousarray(xf.T).astype(BF16_NP)   # [H, T]
    wg = np.asarray(wg, np.float32)
    wu = np.asarray(wu, np.float32)
    wd = np.asarray(wd, np.float32)
    swg = np.asarray(swg, np.float32)
    swu = np.asarray(swu, np.float32)
    swd = np.asarray(swd, np.float32)

    in_maps = []
    idx_r = []
    w_r = []
    for r in range(N_CORES):
        lo, hi = starts[r], starts[r + 1]
        idx = t_sorted[lo:hi]
        idx_r.append(idx)
        w_r.append(w_sorted[lo:hi])
        xg = np.zeros((H, C), dtype=BF16_NP)
        xg[:, :len(idx)] = xfT_bf[:, idx]
        g, q = divmod(r, 2)
        xs_sl = xfT_bf[:, g * TS:(g + 1) * TS]
        off = np.concatenate([[0], np.cumsum(SH_CHUNKS)])
        in_maps.append({
            "xr": _pack_x(xg),
            **{f"xs{i}": _pack_x(xs_sl[:, off[i]:off[i + 1]])
               for i in range(len(SH_CHUNKS))},
            "wgu": np.ascontiguousarray(
                np.concatenate([wg[r], wu[r]], axis=1)).astype(BF16_NP),
            "wd": _pack_down_w(wd[r]),
            "sgu": _pack_gu(swg[:, q * I_R:(q + 1) * I_R],
                            swu[:, q * I_R:(q + 1) * I_R]),
            "sd": _pack_down_w(swd[q * I_R:(q + 1) * I_R, :]),
        })

    res = run_bass_kernel_spmd(nc, in_maps, core_ids=list(range(N_CORES)))
    LAST_RESULT = res

    # ---- host combine: shared partial sums + weighted routed scatter ----
    yT = np.zeros((H, T), np.float32)
    for g in range(4):
        np.add(res.results[2 * g]["ys"].astype(np.float32),
               res.results[2 * g + 1]["ys"].astype(np.float32),
               out=yT[:, g * TS:(g + 1) * TS])
    for r in range(N_CORES):
        n = len(idx_r[r])
        if n:
            yT[:, idx_r[r]] += (res.results[r]["yr"][:, :n]
                                .astype(np.float32)
                                * w_r[r][None, :].astype(np.float32))
    return np.ascontiguousarray(yT.T).reshape(B, S, H).astype(np.float32)


# revision 50
# speedup vs baseline: 1.0072x; 1.0043x over previous
"""Sparse expert-parallel MoE kernel for Trainium2 (8 NeuronCores).

Strategy (hardcoded for the nn_MoE problem: H=1024, E=8, top-k=2, I=1408,
shared-I=2816, T=2*2048=4096 tokens, f32 inputs):

- The gate (softmax top-2) is tiny (0.03% of FLOPs) and is evaluated on the
  host in float64; routing decisions match the f32 reference (min rank-2/3
  score gap for this problem's data is ~4e-5, far above f32 noise).
- Routed experts are EXPERT-PARALLEL with true top-2 sparsity: core r owns
  expert r and computes it only over the tokens routed to it (host-side
  gather, padded to capacity C = max expert load, split into near-equal
  token chunks <= 512).  This is ~4x fewer FLOPs than dense all-expert
  compute.
- The shared expert is sharded 4x2: cores are split into 4 token-groups of
  2; within a group each core owns a 1408-wide half of the 2816 shared
  intermediate dim (11 full 128-tiles -> no partial-tile waste).  Partials
  are summed on the host.
- Combine: host scatter-adds  w_e(t) * expert_e(x_t)  (f32) plus the shared
  partial sums.  No on-device collectives.
- All matmuls run in bf16 with f32 PSUM accumulation (host pre-casts);
  outputs are stored bf16 and accumulated f32 on the host.
- DMA cost is ~(27ns + 29ns/KB) per line (= per partition per transfer),
  descriptors sprayed round-robin over 16 rings, so everything is packed
  into few transfers of long contiguous per-partition lines, SBUF-resident,
  issued in consumption order (the first matmul group only waits for
  ~1.5 MB).  The shared phase runs first: its startup inputs are smallest.

Layouts put features on the partition axis and tokens on the free axis:
    up:   hg[i, t] = sum_h wg[h, i] * xT[h, t]   (lhsT=wg nat., rhs=xT)
    down: eo[h, t] = sum_i wd[i, h] * act[i, t]  (lhsT=wd nat., rhs=act)
"""

import os
import sys

for _p in ("/opt/trn_rl_repo", "/root/.axon_site/_ro/trn_rl_repo"):
    if os.path.isdir(_p) and _p not in sys.path:
        sys.path.insert(0, _p)

import numpy as np

import concourse.bass as bass
import concourse.mybir as mybir
import concourse.tile as tile
from concourse import bacc
from concourse.bass_utils import run_bass_kernel_spmd

F32 = mybir.dt.float32
BF16 = mybir.dt.bfloat16
BF16_NP = mybir.dt.np(mybir.dt.bfloat16)
AX = mybir.AxisListType
ALU = mybir.AluOpType
ACTF = mybir.ActivationFunctionType

H = 1024           # hidden
E = 8              # experts = cores
TOP_K = 2
I_R = 1408         # routed intermediate = shared intermediate half (2816/2)
TS = 1024          # shared-expert tokens per core (4096 / 4 groups)
N_CORES = 8
KC = H // 128      # 8 contraction chunks over hidden
IT_R = I_R // 128  # 11 intermediate tiles (routed and shared-half alike)
TC = 512           # token tile (PSUM bank = 512 f32)
SH_CHUNKS = (512, 512)  # shared-phase token chunking (see build_nc)

LAST_RESULT = None  # BassKernelResults of the most recent run (for profiling)


def _chunks_of(n):
    """Split n into near-equal chunks <= TC (all big enough to keep the
    PE stream-bound rather than weight-load-bound)."""
    nch = max(1, -(-n // TC))
    base, rem = divmod(n, nch)
    return [base + 1] * rem + [base] * (nch - rem)


def build_nc(C, trace_sim=False, silu_via_sigmoid=False):
    """Build the SPMD Bass program (identical on all 8 cores).

    C: routed-token capacity per core (= max expert load for this input).
    silu_via_sigmoid: CoreSim has no Silu LUT; emulate as x*sigmoid(x).
    """
    nc = bacc.Bacc("TRN2", target_bir_lowering=False, debug=False,
                   num_devices=N_CORES)

    # DMA cost is ~27ns + 29ns/KB PER LINE (= per partition per dma_start),
    # descriptors sprayed round-robin over 16 rings -> pack everything into
    # as few, as-long-as-possible contiguous per-partition lines as we can.
    # x inputs packed [128, KC, ntok], loaded with ONE full-tensor DMA.
    xr = nc.dram_tensor("xr", [128, KC, C], BF16, kind="ExternalInput")
    # shared x in per-chunk pieces (contiguous long lines); the first chunk
    # is 384 tokens so the PE starts after only ~1.25 MB of critical DMA,
    # small enough to start early but big enough that sgu tile arrival
    # stays ahead of consumption
    SH_CH = SH_CHUNKS
    xs_d = [nc.dram_tensor(f"xs{i}", [128, KC, n], BF16,
                           kind="ExternalInput")
            for i, n in enumerate(SH_CH)]  # per-chunk pieces, long lines
    # routed gate+up combined: rows h, cols [gate I_R | up I_R] per k-slice
    wgu = nc.dram_tensor("wgu", [H, 2 * I_R], BF16, kind="ExternalInput")
    # down weights packed [128, IT, H] (partition = i % 128)
    wd = nc.dram_tensor("wd", [128, IT_R, H], BF16, kind="ExternalInput")
    # shared gate+up packed [128, IT, 2*KC*128]: per-it DMA, 4KB lines
    sgu = nc.dram_tensor("sgu", [128, IT_R, 2 * KC * 128], BF16,
                         kind="ExternalInput")
    sd = nc.dram_tensor("sd", [128, IT_R, H], BF16, kind="ExternalInput")
    # bf16 stores (host accumulates in f32; |y| ~ O(1), rel-err budget 2e-2)
    yr = nc.dram_tensor("yr", [H, C], BF16, kind="ExternalOutput")
    ys = nc.dram_tensor("ys", [H, TS], BF16, kind="ExternalOutput")

    with tile.TileContext(nc, trace_sim=trace_sim) as tc:
        with (
            tc.tile_pool(name="const", bufs=1) as cpool,
            tc.tile_pool(name="act", bufs=2) as actpool,
            tc.tile_pool(name="tmp", bufs=3) as tpool,
            tc.tile_pool(name="eo", bufs=3) as eopool,
            tc.tile_pool(name="ps_up", bufs=5, space="PSUM") as ps_up,
            tc.tile_pool(name="ps_o", bufs=2, space="PSUM") as ps_o,
            tc.tile_pool(name="ps_w", bufs=1, space="PSUM") as ps_w,
        ):
            # ---- HAM pre-warm: the PE sits idle ~7-14us while the first
            # inputs DMA in; a scratch matmul stream un-throttles the PE
            # clock (1.2 -> 2.4 GHz) before real work arrives, and ends
            # close enough to it that the MID window can't re-throttle ----
            scr = cpool.tile([128, 128], BF16, tag="scr")
            nc.vector.memset(scr[:, :], 0.0)
            ps_scr = ps_w.tile([128, 128], F32, tag="warm")
            for _ in range(80):
                nc.tensor.matmul(ps_scr[:, :], scr[:, :], scr[:, :],
                                 start=True, stop=True)

            # ---- inputs in consumption order; the first psum group needs
            # sgu tile 0 + xs chunk 0, so those go first, then strictly by
            # first-use time (supply rate ~0.45 MB/us must stay ahead of
            # the up-group consumption pace) ----
            sgu_ts = []
            for it in range(IT_R):
                sgut = cpool.tile([128, 2 * KC * 128], BF16, tag=f"sgu{it}")
                sgu_ts.append(sgut)
            xs_ts = []
            for i, n in enumerate(SH_CH):
                xst = cpool.tile([128, KC, n], BF16, tag=f"xs{i}")
                xs_ts.append(xst)
            nc.sync.dma_start(sgu_ts[0][:, :], sgu[:, 0, :])
            nc.sync.dma_start(xs_ts[0][:, :, :], xs_d[0][:, :, :])
            for it in (1, 2, 3):
                nc.sync.dma_start(sgu_ts[it][:, :], sgu[:, it, :])
            nc.sync.dma_start(xs_ts[1][:, :, :], xs_d[1][:, :, :])
            for it in range(4, IT_R):
                nc.sync.dma_start(sgu_ts[it][:, :], sgu[:, it, :])
            sd_t = cpool.tile([128, IT_R, H], BF16, tag="sd")
            nc.sync.dma_start(sd_t[:, :, :], sd[:, :, :])
            # routed inputs (needed ~120us in; stream behind shared ones)
            xr_t = cpool.tile([128, KC, C], BF16, tag="xr")
            nc.sync.dma_start(xr_t[:, :, :], xr[:, :, :])
            wgu_ks = []
            for k in range(KC):
                wguk = cpool.tile([128, 2 * I_R], BF16, tag=f"wgu{k}")
                nc.sync.dma_start(wguk[:, :], wgu[k * 128:(k + 1) * 128, :])
                wgu_ks.append(wguk)
            wd_t = cpool.tile([128, IT_R, H], BF16, tag="wd")
            nc.sync.dma_start(wd_t[:, :, :], wd[:, :, :])

            def swiglu_chunk(xf, n, gate_f, up_f, act_t):
                """act[i, :n] = silu(gate) * up over this token chunk.

                xf: k -> rhs [128, n];  gate_f/up_f: (it, k) -> lhsT block.
                """
                for it in range(IT_R):
                    pg = ps_up.tile([128, TC], F32, tag="up")
                    for k in range(KC):
                        nc.tensor.matmul(pg[:, :n], gate_f(it, k), xf(k),
                                         start=(k == 0), stop=(k == KC - 1))
                    pu = ps_up.tile([128, TC], F32, tag="up")
                    for k in range(KC):
                        nc.tensor.matmul(pu[:, :n], up_f(it, k), xf(k),
                                         start=(k == 0), stop=(k == KC - 1))
                    sa = tpool.tile([128, TC], F32, tag="sa")
                    if silu_via_sigmoid:
                        nc.scalar.activation(sa[:, :n], pg[:, :n],
                                             ACTF.Sigmoid)
                        nc.vector.tensor_mul(sa[:, :n], sa[:, :n], pg[:, :n])
                    else:
                        nc.scalar.activation(sa[:, :n], pg[:, :n], ACTF.Silu)
                    nc.vector.tensor_mul(act_t[:, it, :n], sa[:, :n],
                                         pu[:, :n])

            def down_chunk(act_t, n, dw_t, out_d, t0):
                for hc in range(KC):
                    h0 = hc * 128
                    po = ps_o.tile([128, TC], F32, tag="o")
                    for it in range(IT_R):
                        nc.tensor.matmul(
                            po[:, :n], dw_t[:, it, h0:h0 + 128],
                            act_t[:, it, :n], start=(it == 0),
                            stop=(it == IT_R - 1))
                    eo = eopool.tile([128, TC], BF16)
                    nc.vector.tensor_copy(eo[:, :n], po[:, :n])
                    nc.sync.dma_start(out_d[h0:h0 + 128, t0:t0 + n], eo[:, :n])

            # ---- shared expert half over this core's token group ----
            t0 = 0
            for ci, n in enumerate(SH_CH):
                act_t = actpool.tile([128, IT_R, TC], BF16, tag="act")
                xst = xs_ts[ci]
                swiglu_chunk(
                    lambda k: xst[:, k, :],
                    n,
                    lambda it, k: sgu_ts[it][:, k * 128:(k + 1) * 128],
                    lambda it, k: sgu_ts[it][:, (KC + k) * 128:
                                             (KC + k + 1) * 128], act_t)
                down_chunk(act_t, n, sd_t, ys, t0)
                t0 += n

            # ---- routed expert over gathered tokens ----
            t0 = 0
            for n in _chunks_of(C):
                act_t = actpool.tile([128, IT_R, TC], BF16, tag="act")
                c0 = t0
                swiglu_chunk(
                    lambda k: xr_t[:, k, c0:c0 + n],
                    n,
                    lambda it, k: wgu_ks[k][:, it * 128:(it + 1) * 128],
                    lambda it, k: wgu_ks[k][:, I_R + it * 128:
                                            I_R + (it + 1) * 128], act_t)
                down_chunk(act_t, n, wd_t, yr, t0)
                t0 += n

    nc.compile()
    return nc


def _route_host(xf, gate_w):
    """Replicate the reference MoEGate exactly (float64 for determinism)."""
    logits = xf.astype(np.float64) @ gate_w.astype(np.float64).T
    m = logits.max(axis=-1, keepdims=True)
    ex = np.exp(logits - m)
    sc = ex / ex.sum(axis=-1, keepdims=True)
    topi = np.argsort(-sc, axis=-1, kind="stable")[:, :TOP_K]   # ties: low idx
    topw = np.take_along_axis(sc, topi, axis=-1)
    topw = topw / (topw.sum(axis=-1, keepdims=True) + 1e-20)    # SCALE = 1.0
    return topi, topw


def _pack_x(xT_bf):
    """[H, ntok] -> [128, KC, ntok] partition-major pack."""
    n = xT_bf.shape[1]
    return np.ascontiguousarray(
        xT_bf.reshape(KC, 128, n).transpose(1, 0, 2))


def _pack_x_cm(xT_bf):
    """[H, TS] -> [128, TS//TC, KC, TC] chunk-major pack."""
    a = xT_bf.reshape(KC, 128, TS // TC, TC)
    return np.ascontiguousarray(a.transpose(1, 2, 0, 3))


def _pack_up_w(w):
    """[H, I_R] -> [128, IT, KC*128] pack."""
    a = np.ascontiguousarray(w).astype(BF16_NP)
    a = a.reshape(KC, 128, IT_R, 128).transpose(1, 2, 0, 3)
    return np.ascontiguousarray(a.reshape(128, IT_R, KC * 128))


def _pack_gu(g, u):
    """Two [H, I_R] up-weights -> [128, IT, 2*KC*128] interleaved pack."""
    gp = _pack_up_w(g)
    up = _pack_up_w(u)
    return np.ascontiguousarray(
        np.concatenate([gp[:, :, None, :], up[:, :, None, :]],
                       axis=2).reshape(128, IT_R, 2 * KC * 128))


def _pack_down_w(w):
    """[I_R, H] -> [128, IT, H] pack (partition = i % 128)."""
    a = np.ascontiguousarray(w).astype(BF16_NP)
    return np.ascontiguousarray(a.reshape(IT_R, 128, H).transpose(1, 0, 2))


_NC_CACHE = {}


def kernel(x, gate_w, wg, wu, wd, swg, swu, swd):
    global LAST_RESULT
    x = np.asarray(x, np.float32)
    B, S, _ = x.shape
    T = B * S
    xf = x.reshape(T, H)

    # ---- host gate + dispatch ----
    topi, topw = _route_host(xf, np.asarray(gate_w, np.float32))
    e_ids = topi.ravel()
    t_ids = np.repeat(np.arange(T), TOP_K)
    w_all = topw.ravel()
    order = np.argsort(e_ids, kind="stable")
    e_sorted = e_ids[order]
    t_sorted = t_ids[order]
    w_sorted = w_all[order]
    counts = np.bincount(e_sorted, minlength=E)
    starts = np.concatenate([[0], np.cumsum(counts)])
    C = max(128, int(counts.max()))

    if C not in _NC_CACHE:
        _NC_CACHE[C] = build_nc(C)
    nc = _NC_CACHE[C]

    xfT_bf = np.ascontiguousarray(xf.T).astype(BF16_NP)   # [H, T]
    wg = np.asarray(wg, np.float32)
    wu = np.asarray(wu, np.float32)
    wd = np.asarray(wd, np.float32)
    swg = np.asarray(swg, np.float32)
    swu = np.asarray(swu, np.float32)
    swd = np.asarray(swd, np.float32)

    in_maps = []
    idx_r = []
    w_r = []
    for r in range(N_CORES):
        lo, hi = starts[r], starts[r + 1]
        idx = t_sorted[lo:hi]
        idx_r.append(idx)
        w_r.append(w_sorted[lo:hi])
        xg = np.zeros((H, C), dtype=BF16_NP)
        xg[:, :len(idx)] = xfT_bf[:, idx]
        g, q = divmod(r, 2)
        xs_sl = xfT_bf[:, g * TS:(g + 1) * TS]
        off = np.concatenate([[0], np.cumsum(SH_CHUNKS)])
        in_maps.append({
            "xr": _pack_x(xg),
            **{f"xs{i}": _pack_x(xs_sl[:, off[i]:off[i + 1]])
               for i in range(len(SH_CHUNKS))},
            "wgu": np.ascontiguousarray(
                np.concatenate([wg[r], wu[r]], axis=1)).astype(BF16_NP),
            "wd": _pack_down_w(wd[r]),
            "sgu": _pack_gu(swg[:, q * I_R:(q + 1) * I_R],
                            swu[:, q * I_R:(q + 1) * I_R]),
            "sd": _pack_down_w(swd[q * I_R:(q + 1) * I_R, :]),
        })

    res = run_bass_kernel_spmd(nc, in_maps, core_ids=list(range(N_CORES)))
    LAST_RESULT = res

    # ---- host combine: shared partial sums + weighted routed scatter ----
    yT = np.zeros((H, T), np.float32)
    for g in range(4):
        np.add(res.results[2 * g]["ys"].astype(np.float32),
               res.results[2 * g + 1]["ys"].astype(np.float32),
               out=yT[:, g * TS:(g + 1) * TS])
    for r in range(N_CORES):
        n = len(idx_r[r])
        if n:
            yT[:, idx_r[r]] += (res.results[r]["yr"][:, :n]
                                .astype(np.float32)
                                * w_r[r][None, :].astype(np.float32))
    return np.ascontiguousarray(yT.T).reshape(B, S, H).astype(np.float32)
